# revision 1
# baseline (speedup 1.0000x reference)
"""Multi-head causal attention (B=4, S=2048, D=1024, H=16) on 8 TRN2 NeuronCores.

Sharding: core c handles batch b = c//2 and head-group hg = c%2 (8 heads each).
Each core computes Q/K/V projections for its (batch, head-group), causal
attention, and a partial output projection over its 512 head-dims.  The host
sums the two partials per batch and adds b_o.  No collectives.

Device-side layout choices:
  - x is passed transposed (xT [D, S]) so projection matmuls contract over
    partitions directly.
  - Q and K are produced transposed (QT/KT [dq, S]); scores are computed
    transposed (S^T [kpos, q]) which makes the softmax denominator a matmul
    with a ones-column (no partition reductions anywhere).
  - No max-subtraction in softmax: scaled scores are ~N(0,1), exp is safe.
  - P (=exp(scores)) and V are bf16 for the P@V matmul; everything else is
    float32r (full-rate fp32 on the PE).
"""

import sys
import os

sys.path.insert(0, "/opt/trn_rl_repo")

import numpy as np

import concourse.bacc as bacc
import concourse.mybir as mybir
import concourse.tile as tile
from concourse.bass_utils import run_bass_kernel_spmd

# The ACT table-load pass resolves each activation to the first table set
# containing it, which puts Exp (exp_and_others) and Ln
# (natural_log_exp_and_others) in different sets and reloads tables at every
# softmax normalization.  Restrict Exp/Ln to the one set that holds both so
# the whole kernel runs off a single table load.
_orig_get_tables = bacc.get_activation_tables


def _patched_tables(arch):
    t = _orig_get_tables(arch)
    for name, fns in t.items():
        if name != "natural_log_exp_and_others":
            fns.discard(mybir.ActivationFunctionType.Exp)
            fns.discard(mybir.ActivationFunctionType.Ln)
    return t


bacc.get_activation_tables = _patched_tables

B, S, D, H = 4, 2048, 1024, 16
DK = D // H          # 64
HH = H // 2          # 8 heads per core
HD = HH * DK         # 512 head-dims per core
N_CORES = 8

F32 = mybir.dt.float32
F32R = mybir.dt.float32r
BF16 = mybir.dt.bfloat16
F16 = mybir.dt.float16

SCALE = 1.0 / np.sqrt(DK)


def act_reciprocal(nc, out, in_):
    """Reciprocal on the ACT LUT (~1e-3 rel err, fine for softmax denoms).

    bass's activation() helper refuses Reciprocal for accuracy reasons;
    emit the instruction directly."""
    eng = nc.scalar
    ins = [eng.lower_ap(in_)]
    for v in (0.0, 1.0, 0.0):  # bias, scale, alpha
        ins.append(mybir.ImmediateValue(dtype=mybir.dt.float32, value=v))
    return eng.add_instruction(mybir.InstActivation(
        name=nc.get_next_instruction_name(),
        func=mybir.ActivationFunctionType.Reciprocal,
        ins=ins, outs=[eng.lower_ap(out)]))


def build_nc(s=S, interleave_pairs=True):
    """Build the per-core SPMD program.  `s` is the sequence length (tunable
    for small-scale simulation; must be a multiple of 512)."""
    assert s % 512 == 0
    n_qb = s // 512          # 512-wide q blocks
    n_t128 = s // 128        # 128-wide token tiles
    n_dt = D // 128          # din tiles (8)

    nc = bacc.Bacc("TRN2", target_bir_lowering=False, debug=False,
                   num_devices=N_CORES)

    xT = nc.dram_tensor("xT", [D, s], F16, kind="ExternalInput")
    wqT = nc.dram_tensor("wqT", [D, HD], F16, kind="ExternalInput")
    wkT = nc.dram_tensor("wkT", [D, HD], F16, kind="ExternalInput")
    wvT = nc.dram_tensor("wvT", [D, HD], F16, kind="ExternalInput")
    woT = nc.dram_tensor("woT", [HD, D], F16, kind="ExternalInput")
    out = nc.dram_tensor("out", [s, D], F32, kind="ExternalOutput")

    with tile.TileContext(nc) as tc:
        with tc.tile_pool(name="persist", bufs=1) as persist, \
             tc.tile_pool(name="wload", bufs=16) as wload, \
             tc.tile_pool(name="xtb", bufs=12) as xtb_pool, \
             tc.tile_pool(name="pT", bufs=40) as pT_pool, \
             tc.tile_pool(name="aoT", bufs=8) as aoT_pool, \
             tc.tile_pool(name="rb", bufs=4) as rb_pool, \
             tc.tile_pool(name="outsb", bufs=2) as out_pool, \
             tc.tile_pool(name="xsl", bufs=8) as xsl_pool, \
             tc.tile_pool(name="spsum", bufs=3, space="PSUM") as spsum, \
             tc.tile_pool(name="upsum", bufs=3, space="PSUM") as upsum, \
             tc.tile_pool(name="opsum", bufs=2, space="PSUM") as opsum:

            # Persistent SBUF arrays (live for the whole kernel).
            qt_sb = [persist.tile([128, s], F16, tag=f"qt{d}", name=f"qt{d}") for d in range(HD // 128)]
            # Per-head K^T tiles, zero-padded to 128 contraction rows: head h
            # occupies rows (h%2)*64..(h%2)*64+63, the other 64 rows are zero.
            # Scores matmuls can then use full 128x128 PE mode (the zero rows
            # multiply the paired head's Q rows harmlessly) -- avoiding tiling
            # mode switches, which drain the PE between instructions.
            kt_sb = [persist.tile([128, s], F16, tag=f"kt{h}", name=f"kt{h}") for h in range(HH)]
            for h in range(HH):
                z = (1 - h % 2) * 64
                nc.vector.memset(kt_sb[h][z:z + 64, :], 0.0)
            # V tiles hold [t, head, 2*dk]: cols 0-63 are V, cols 64-127 are
            # 1.0.  As the AV stationary this makes the matmul emit U^T on
            # psum rows 0-63 and the softmax denominator on rows 64-127.
            v_sb = [persist.tile([128, HH, 2 * DK], F16, tag=f"v{t}", name=f"v{t}") for t in range(n_t128)]
            wo_sb = [persist.tile([128, D], F16, tag=f"wo{d}", name=f"wo{d}") for d in range(HD // 128)]
            wv_sb = [persist.tile([128, HD], F16, tag=f"wv{i}", name=f"wv{i}") for i in range(n_dt)]

            # Weights first: they gate the first matmuls.
            w_tiles = {}
            for wdram, wkey in ((wqT, "q"), (wkT, "k")):
                for i in range(n_dt):
                    wt = wload.tile([128, HD], F16, tag="w", name="w")
                    nc.sync.dma_start(out=wt[:], in_=wdram[i * 128:(i + 1) * 128, :])
                    w_tiles[(wkey, i)] = wt
            for i in range(n_dt):
                nc.sync.dma_start(out=wv_sb[i][:], in_=wvT[i * 128:(i + 1) * 128, :])
            for d in range(HD // 128):
                nc.sync.dma_start(out=wo_sb[d][:], in_=woT[d * 128:(d + 1) * 128, :])

            def emit_proj_chains(tb, dqs):
                """Q^T and K^T projection chains for one 512-token block and
                the given dq tiles, streaming x straight from DRAM."""
                xs = []
                for i in range(n_dt):
                    t = xtb_pool.tile([128, 512], F16, tag="xtb", name="xtb")
                    nc.sync.dma_start(
                        out=t[:], in_=xT[i * 128:(i + 1) * 128,
                                         tb * 512:(tb + 1) * 512])
                    xs.append(t)
                for dq in dqs:
                    for wkey, is_k in (("q", False), ("k", True)):
                        ps = opsum.tile([128, 512], F32, tag="op", name="pp")
                        for i in range(n_dt):
                            nc.tensor.matmul(
                                ps[:],
                                lhsT=w_tiles[(wkey, i)][:, dq * 128:(dq + 1) * 128],
                                rhs=xs[i][:],
                                start=(i == 0), stop=(i == n_dt - 1),
                            )
                        if is_k:
                            for e in (0, 1):
                                nc.vector.tensor_copy(
                                    out=kt_sb[2 * dq + e][e * 64:(e + 1) * 64,
                                                          tb * 512:(tb + 1) * 512],
                                    in_=ps[e * 64:(e + 1) * 64, :])
                        else:
                            nc.vector.tensor_copy(
                                out=qt_sb[dq][:, tb * 512:(tb + 1) * 512], in_=ps[:])

            def emit_v_chain(tb):
                """V projection for one 128-token tile, spliced into the
                attention stream just before the q-block that needs it."""
                vp = opsum.tile([128, 512], F32, tag="op", name="vp")
                for i in range(n_dt):
                    xs = xsl_pool.tile([128, 128], F16, tag="xs", name="xs")
                    nc.sync.dma_start(
                        out=xs[:],
                        in_=xT[i * 128:(i + 1) * 128, tb * 128:(tb + 1) * 128])
                    nc.tensor.matmul(
                        vp[:], lhsT=xs[:], rhs=wv_sb[i][:],
                        start=(i == 0), stop=(i == n_dt - 1),
                    )
                # ones columns for the denominator, then V data (cast fp16)
                nc.vector.memset(v_sb[tb][:, :, DK:2 * DK], 1.0)
                nc.vector.tensor_copy(
                    out=v_sb[tb][:, :, 0:DK],
                    in_=vp[:].rearrange("p (h k) -> p h k", h=HH))

            def emit_score_kt(qb, hp, kt, pT):
                lo = max(kt - 4 * qb, 0) * 128
                for hh in (0, 1):
                    sp = spsum.tile([128, 512], F32, tag="sp", name="sp")
                    nc.tensor.matmul(
                        sp[:, lo:512],
                        lhsT=kt_sb[2 * hp + hh][:, kt * 128:(kt + 1) * 128],
                        rhs=qt_sb[hp][:, qb * 512 + lo:(qb + 1) * 512],
                        start=True, stop=True,
                    )
                    p = pT_pool.tile([128, 512], F16, tag="p", name="p")
                    if lo > 0:
                        # below-diagonal columns: P must be exact zeros
                        # (they are read by the full-width AV matmul)
                        nc.gpsimd.memset(p[:, 0:lo], 0.0)
                    nc.scalar.activation(
                        out=p[:, lo:512], in_=sp[:, lo:512],
                        func=mybir.ActivationFunctionType.Exp,
                        scale=float(SCALE))
                    if kt >= 4 * qb:
                        # zero strict-upper (kpos > q) region of the
                        # diagonal-crossing tile
                        nc.gpsimd.affine_select(
                            out=p[:, lo:512], in_=p[:, lo:512],
                            compare_op=mybir.AluOpType.is_ge,
                            fill=0.0, base=0, channel_multiplier=-1,
                            pattern=[[1, 512 - lo]])
                    pT[(kt, hh)] = p

            def emit_pair(cur, nxt, pT_cur, pT_next):
                """Interleave next pair's scores with current pair's AV
                chains at kt granularity: the PE gets AV matmuls to run
                while the ACT engine works through the scores' exps."""
                nkt_cur = 4 * cur[0] + 4 if cur else 0
                nkt_nxt = 4 * nxt[0] + 4 if nxt else 0
                u = {}
                ao = None
                if cur:
                    ao = aoT_pool.tile([128, 512], F16, tag="aoT", name="aoT")
                    for hh in (0, 1):
                        u[hh] = upsum.tile([128, 512], F32, tag="u", name="u")
                for kt in range(max(nkt_cur, nkt_nxt)):
                    if kt < nkt_nxt:
                        emit_score_kt(nxt[0], nxt[1], kt, pT_next)
                    if kt < nkt_cur:
                        for hh in (0, 1):
                            nc.tensor.matmul(
                                u[hh][:],
                                lhsT=v_sb[kt][:, 2 * cur[1] + hh, :],
                                rhs=pT_cur[(kt, hh)][:],
                                start=(kt == 0), stop=(kt == nkt_cur - 1),
                            )
                if cur:
                    for hh in (0, 1):
                        # rows 0-63: U^T; rows 64-127: denominator bcast.
                        # 1/l = exp(-ln(l)): ln and exp share one ACT table
                        # set, so no table reloads.
                        rb = rb_pool.tile([128, 512], F32, tag="rb", name="rb")
                        nc.scalar.activation(
                            out=rb[64:128, :], in_=u[hh][64:128, :],
                            func=mybir.ActivationFunctionType.Ln)
                        nc.scalar.activation(
                            out=rb[64:128, :], in_=rb[64:128, :],
                            func=mybir.ActivationFunctionType.Exp, scale=-1.0)
                        nc.vector.tensor_mul(
                            out=ao[hh * 64:(hh + 1) * 64, :],
                            in0=u[hh][0:64, :], in1=rb[64:128, :])
                return ao

            def emit_oproj(qb, ao_pairs):
                for qt_l in range(4):
                    qt = 4 * qb + qt_l
                    osb = out_pool.tile([128, D], F32, tag="osb", name="osb")
                    for half in range(2):
                        op = opsum.tile([128, 512], F32, tag="op", name="op")
                        for hp in range(HH // 2):
                            nc.tensor.matmul(
                                op[:],
                                lhsT=ao_pairs[hp][:, qt_l * 128:(qt_l + 1) * 128],
                                rhs=wo_sb[hp][:, half * 512:(half + 1) * 512],
                                start=(hp == 0), stop=(hp == 3),
                            )
                        nc.vector.tensor_copy(
                            out=osb[:, half * 512:(half + 1) * 512], in_=op[:])
                    nc.sync.dma_start(
                        out=out[qt * 128:(qt + 1) * 128, :], in_=osb[:])

            # Demand-driven schedule: projections for q-block tb are emitted
            # inside q-block tb-1's pairs; V chains just before the block
            # needing them; out-projections three pairs after their block.
            # dq-interleaved first block so pair (0,0) unblocks after two
            # chains.
            for dq in range(HD // 128):
                emit_proj_chains(0, [dq])
            for tb in range(4):
                emit_v_chain(tb)
            pairs = [(qb, hp) for qb in range(n_qb) for hp in range(HH // 2)]
            pT_next = {}
            emit_pair(None, pairs[0], None, pT_next)
            ao_by_qb = {qb: [] for qb in range(n_qb)}
            oproj_queue = []
            for i, (qb, hp) in enumerate(pairs):
                pT_cur, pT_next = pT_next, {}
                nxt = pairs[i + 1] if i + 1 < len(pairs) else None
                if nxt and nxt[1] == 0 and nxt[0] > 0:
                    for tb in range(4 * nxt[0], 4 * nxt[0] + 4):
                        emit_v_chain(tb)
                if oproj_queue and oproj_queue[0][1] <= i:
                    oqb, _ = oproj_queue.pop(0)
                    emit_oproj(oqb, ao_by_qb.pop(oqb))
                ao_by_qb[qb].append(emit_pair((qb, hp), nxt, pT_cur, pT_next))
                if qb + 1 < n_qb:
                    # projections for the next q-block, two dq chains per pair
                    emit_proj_chains(qb + 1, [hp])
                if hp == HH // 2 - 1:
                    oproj_queue.append((qb, i + 3))
            for oqb, _ in oproj_queue:
                emit_oproj(oqb, ao_by_qb.pop(oqb))

    nc.compile()
    return nc


_NC_CACHE = {}


def _get_nc(s=S):
    if s not in _NC_CACHE:
        _NC_CACHE[s] = build_nc(s)
    return _NC_CACHE[s]


def make_in_maps(x, w_q, w_k, w_v, w_o, s=S):
    """Host-side sharding: returns the 8 per-core input maps."""
    x = np.ascontiguousarray(np.asarray(x, dtype=np.float32))
    w_q = np.asarray(w_q, dtype=np.float32)
    w_k = np.asarray(w_k, dtype=np.float32)
    w_v = np.asarray(w_v, dtype=np.float32)
    w_o = np.asarray(w_o, dtype=np.float32)

    xTs = [np.ascontiguousarray(x[b].T.astype(np.float16)) for b in range(B)]
    wqTs = [np.ascontiguousarray(w_q[hg * HD:(hg + 1) * HD, :].T.astype(np.float16)) for hg in range(2)]
    wkTs = [np.ascontiguousarray(w_k[hg * HD:(hg + 1) * HD, :].T.astype(np.float16)) for hg in range(2)]
    wvTs = [np.ascontiguousarray(w_v[hg * HD:(hg + 1) * HD, :].T.astype(np.float16)) for hg in range(2)]
    woTs = [np.ascontiguousarray(w_o[:, hg * HD:(hg + 1) * HD].T.astype(np.float16)) for hg in range(2)]

    in_maps = []
    for c in range(N_CORES):
        b, hg = c // 2, c % 2
        in_maps.append({
            "xT": xTs[b], "wqT": wqTs[hg], "wkT": wkTs[hg],
            "wvT": wvTs[hg], "woT": woTs[hg],
        })
    return in_maps


def kernel(x, w_q, w_k, w_v, w_o, b_o):
    nc = _get_nc(S)
    in_maps = make_in_maps(x, w_q, w_k, w_v, w_o, s=S)
    res = run_bass_kernel_spmd(nc, in_maps, core_ids=list(range(N_CORES)))
    b_o = np.asarray(b_o, dtype=np.float32)
    outp = np.empty((B, S, D), dtype=np.float32)
    for b in range(B):
        outp[b] = res.results[2 * b]["out"] + res.results[2 * b + 1]["out"] + b_o
    return outp



# revision 6
# speedup vs baseline: 1.0682x; 1.0682x over previous
"""Multi-head causal attention (B=4, S=2048, D=1024, H=16) on 8 TRN2 NeuronCores.

Sharding: core c handles batch b = c//2 and head-group hg = c%2 (8 heads each).
Each core computes Q/K/V projections for its (batch, head-group), causal
attention, and a partial output projection over its 512 head-dims.  The host
sums the two partials per batch and adds b_o.  No collectives.

Device-side layout choices:
  - x is passed transposed (xT [D, S]), loaded once into SBUF and reused by
    all projection chains.
  - Q and K are produced transposed (QT/KT [dq, S]); scores are computed
    transposed (S^T [kpos, q]).  K^T tiles keep the natural head-pair packing
    (head 2d in rows 0-63, head 2d+1 in rows 64-127) and the two heads'
    score matmuls run CONCURRENTLY on the PE via row tiling (64-row
    contraction each, tile_position (0,0) / (64,0)).
  - Scores for one kt tile and both heads land in one 2-bank [128, 1024]
    PSUM tile, so a single ACT exp covers both heads (halves ACT instruction
    overhead, the co-bottleneck).
  - No max-subtraction in softmax: scaled scores are ~N(0,1), exp is safe.
  - AV matmuls trim the below-diagonal (always-zero) query range instead of
    memsetting P; only the 128-wide diagonal square needs affine_select.
  - U (64 value dims + 64 denominator-broadcast rows, via ones columns in V)
    is copied out of PSUM to SBUF in fp16 right away so only 2 PSUM banks of
    accumulators are needed; 1/l = exp(-ln(l)) runs on the SBUF copy off the
    critical path (ln and exp share one ACT table set -> no table reloads).
"""

import sys
import os

sys.path.insert(0, "/opt/trn_rl_repo")

import numpy as np

import concourse.bacc as bacc
import concourse.mybir as mybir
import concourse.tile as tile
from concourse.bass_utils import run_bass_kernel_spmd

# The ACT table-load pass resolves each activation to the first table set
# containing it, which puts Exp (exp_and_others) and Ln
# (natural_log_exp_and_others) in different sets and reloads tables at every
# softmax normalization.  Restrict Exp/Ln to the one set that holds both so
# the whole kernel runs off a single table load.
_orig_get_tables = bacc.get_activation_tables


def _patched_tables(arch):
    t = _orig_get_tables(arch)
    for name, fns in t.items():
        if name != "natural_log_exp_and_others":
            fns.discard(mybir.ActivationFunctionType.Exp)
            fns.discard(mybir.ActivationFunctionType.Ln)
    return t


bacc.get_activation_tables = _patched_tables

B, S, D, H = 4, 2048, 1024, 16
DK = D // H          # 64
HH = H // 2          # 8 heads per core
HD = HH * DK         # 512 head-dims per core
N_CORES = 8

F32 = mybir.dt.float32
F16 = mybir.dt.float16

SCALE = 1.0 / np.sqrt(DK)


def build_nc(s=S):
    """Build the per-core SPMD program.  `s` is the sequence length (tunable
    for small-scale simulation; must be a multiple of 1024)."""
    assert s % 1024 == 0
    n_qb = s // 512          # 512-wide q blocks
    n_t128 = s // 128        # 128-wide token tiles
    n_tbb = s // 1024        # 1024-wide token blocks (projection chains)
    n_dt = D // 128          # din tiles (8)

    nc = bacc.Bacc("TRN2", target_bir_lowering=False, debug=False,
                   num_devices=N_CORES)

    xT = nc.dram_tensor("xT", [D, s], F16, kind="ExternalInput")
    wqT = nc.dram_tensor("wqT", [D, HD], F16, kind="ExternalInput")
    wkT = nc.dram_tensor("wkT", [D, HD], F16, kind="ExternalInput")
    wvT = nc.dram_tensor("wvT", [D, HD], F16, kind="ExternalInput")
    woT = nc.dram_tensor("woT", [HD, D], F16, kind="ExternalInput")
    out = nc.dram_tensor("out", [s, D], F16, kind="ExternalOutput")

    with tile.TileContext(nc) as tc:
        with tc.tile_pool(name="persist", bufs=1) as persist, \
             tc.tile_pool(name="pT", bufs=20) as pT_pool, \
             tc.tile_pool(name="usb", bufs=3) as usb_pool, \
             tc.tile_pool(name="rb", bufs=2) as rb_pool, \
             tc.tile_pool(name="aoT", bufs=8) as aoT_pool, \
             tc.tile_pool(name="outsb", bufs=2) as out_pool, \
             tc.tile_pool(name="bigp", bufs=3, space="PSUM") as bigp, \
             tc.tile_pool(name="upsum", bufs=2, space="PSUM") as upsum:

            # Persistent SBUF arrays (live for the whole kernel).
            # Q^T / K^T per head-pair: head 2d in rows 0-63, 2d+1 in 64-127.
            qt_sb = [persist.tile([128, s], F16, tag=f"qt{d}", name=f"qt{d}") for d in range(HD // 128)]
            kt_sb = [persist.tile([128, s], F16, tag=f"kt{d}", name=f"kt{d}") for d in range(HD // 128)]
            # V tiles hold [t, head, 2*dk]: cols 0-63 are V, cols 64-127 are
            # 1.0.  As the AV stationary this makes the matmul emit U^T on
            # psum rows 0-63 and the softmax denominator on rows 64-127.
            v_sb = [persist.tile([128, HH, 2 * DK], F16, tag=f"v{t}", name=f"v{t}") for t in range(n_t128)]
            wo_sb = [persist.tile([128, D], F16, tag=f"wo{d}", name=f"wo{d}") for d in range(HD // 128)]
            wv_sb = [persist.tile([128, HD], F16, tag=f"wv{i}", name=f"wv{i}") for i in range(n_dt)]
            # x resident: [tbb][i] -> [128, 1024] (din tile i, token block tbb)
            x_sb = [[persist.tile([128, 1024], F16, tag=f"x{tbb}_{i}", name=f"x{tbb}_{i}")
                     for i in range(n_dt)] for tbb in range(n_tbb)]
            # Q/K weights in [128, 256] column slices (dq pairs) so the first
            # chain is gated on only 8 small DMAs.
            w_tiles = {}
            for wkey in ("q", "k"):
                for i in range(n_dt):
                    for dqp in range(HD // 256):
                        w_tiles[(wkey, i, dqp)] = persist.tile(
                            [128, 256], F16, tag=f"w{wkey}{i}_{dqp}",
                            name=f"w{wkey}{i}_{dqp}")

            # Ones columns of the V tiles (written once; V chains only write
            # cols 0-63).  On gpsimd to keep the DVE free.
            for t in range(n_t128):
                nc.gpsimd.memset(v_sb[t][:, :, DK:2 * DK], 1.0)

            # DMAs in first-use order: (wq, dqp=0) gates the very first
            # chain; x block 0 tiles trickle in alongside it.
            for i in range(n_dt):
                nc.sync.dma_start(
                    out=w_tiles[("q", i, 0)][:],
                    in_=wqT[i * 128:(i + 1) * 128, 0:256])
                nc.sync.dma_start(
                    out=x_sb[0][i][:], in_=xT[i * 128:(i + 1) * 128, 0:1024])
            for i in range(n_dt):
                nc.sync.dma_start(
                    out=w_tiles[("k", i, 0)][:],
                    in_=wkT[i * 128:(i + 1) * 128, 0:256])
            for dqp in range(1, HD // 256):
                for wkey, wdram in (("q", wqT), ("k", wkT)):
                    for i in range(n_dt):
                        nc.sync.dma_start(
                            out=w_tiles[(wkey, i, dqp)][:],
                            in_=wdram[i * 128:(i + 1) * 128,
                                      dqp * 256:(dqp + 1) * 256])
            for i in range(n_dt):
                nc.sync.dma_start(out=wv_sb[i][:], in_=wvT[i * 128:(i + 1) * 128, :])
            for d in range(HD // 128):
                nc.sync.dma_start(out=wo_sb[d][:], in_=woT[d * 128:(d + 1) * 128, :])
            for tbb in range(1, n_tbb):
                for i in range(n_dt):
                    nc.sync.dma_start(
                        out=x_sb[tbb][i][:],
                        in_=xT[i * 128:(i + 1) * 128, tbb * 1024:(tbb + 1) * 1024])

            def emit_qk_chain(tbb, dq, is_k):
                """Q^T or K^T projection for one 1024-token block and one
                head-pair dq, off the resident x tiles."""
                wkey = "k" if is_k else "q"
                ps = bigp.tile([128, 1024], F32, tag="big", name="qk")
                for half in range(2):
                    for i in range(n_dt):
                        w = w_tiles[(wkey, i, dq // 2)][:, (dq % 2) * 128:(dq % 2 + 1) * 128]
                        nc.tensor.matmul(
                            ps[:, half * 512:(half + 1) * 512],
                            lhsT=w,
                            rhs=x_sb[tbb][i][:, half * 512:(half + 1) * 512],
                            start=(i == 0), stop=(i == n_dt - 1),
                        )
                dst = kt_sb[dq] if is_k else qt_sb[dq]
                nc.vector.tensor_copy(
                    out=dst[:, tbb * 1024:(tbb + 1) * 1024], in_=ps[:])

            def emit_v_chain(tb):
                """V projection for one 128-token tile, spliced into the
                attention stream just before the q-block that needs it."""
                vp = bigp.tile([128, 1024], F32, tag="big", name="vp")
                for i in range(n_dt):
                    xs = x_sb[tb // 8][i][:, (tb % 8) * 128:(tb % 8 + 1) * 128]
                    nc.tensor.matmul(
                        vp[:, 0:512], lhsT=xs, rhs=wv_sb[i][:],
                        start=(i == 0), stop=(i == n_dt - 1),
                    )
                nc.vector.tensor_copy(
                    out=v_sb[tb][:, :, 0:DK],
                    in_=vp[:, 0:512].rearrange("p (h k) -> p h k", h=HH))

            def emit_score_kt(qb, hp, kt, pT):
                """Scores + exp for one kt tile, BOTH heads of the pair:
                row-tiled matmuls (64-contraction each) into one 2-bank psum
                tile, one exp over both."""
                lo = max(kt - 4 * qb, 0) * 128
                sp = bigp.tile([128, 1024], F32, tag="big", name="sp")
                for hh in (0, 1):
                    nc.tensor.matmul(
                        sp[:, hh * 512 + lo:(hh + 1) * 512],
                        lhsT=kt_sb[hp][hh * 64:(hh + 1) * 64,
                                       kt * 128:(kt + 1) * 128],
                        rhs=qt_sb[hp][hh * 64:(hh + 1) * 64,
                                      qb * 512 + lo:(qb + 1) * 512],
                        start=True, stop=True,
                    )
                p = pT_pool.tile([128, 1024], F16, tag="p", name="p")
                nc.scalar.activation(
                    out=p[:, lo:1024], in_=sp[:, lo:1024],
                    func=mybir.ActivationFunctionType.Exp,
                    scale=float(SCALE))
                if kt >= 4 * qb:
                    # zero strict-upper (kpos > q) region of the 128-wide
                    # diagonal square, per head
                    for hh in (0, 1):
                        nc.gpsimd.affine_select(
                            out=p[:, hh * 512 + lo:hh * 512 + lo + 128],
                            in_=p[:, hh * 512 + lo:hh * 512 + lo + 128],
                            compare_op=mybir.AluOpType.is_ge,
                            fill=0.0, base=0, channel_multiplier=-1,
                            pattern=[[1, 128]])
                pT[kt] = (p, lo)

            def emit_pair(cur, nxt, pT_cur, pT_next):
                """Interleave next pair's scores with current pair's AV
                chains at kt granularity: the PE gets AV matmuls to run
                while the ACT engine works through the scores' exps."""
                nkt_cur = 4 * cur[0] + 4 if cur else 0
                nkt_nxt = 4 * nxt[0] + 4 if nxt else 0
                u = {}
                if cur:
                    for hh in (0, 1):
                        u[hh] = upsum.tile([128, 512], F32, tag="u", name="u")
                for kt in range(max(nkt_cur, nkt_nxt)):
                    if kt < nkt_nxt:
                        emit_score_kt(nxt[0], nxt[1], kt, pT_next)
                    if kt < nkt_cur:
                        p, lo = pT_cur[kt]
                        for hh in (0, 1):
                            nc.tensor.matmul(
                                u[hh][:, lo:512],
                                lhsT=v_sb[kt][:, 2 * cur[1] + hh, :],
                                rhs=p[:, hh * 512 + lo:(hh + 1) * 512],
                                start=(kt == 0), stop=(kt == nkt_cur - 1),
                            )
                if not cur:
                    return None
                # Evacuate U to SBUF fp16 (rows 64-127: PSUM reads can be
                # partition-shifted, SBUF-SBUF operands can't) and take
                # ln of the denominators straight from PSUM; both free the
                # psum bank quickly for the next pair.
                usb = usb_pool.tile([128, 1024], F16, tag="usb", name="usb")
                rb = rb_pool.tile([128, 1024], F32, tag="rb", name="rb")
                for hh in (0, 1):
                    nc.vector.tensor_copy(
                        out=usb[64:128, hh * 512:(hh + 1) * 512],
                        in_=u[hh][0:64, :])
                    nc.scalar.activation(
                        out=rb[64:128, hh * 512:(hh + 1) * 512],
                        in_=u[hh][64:128, :],
                        func=mybir.ActivationFunctionType.Ln)
                # 1/l = exp(-ln(l)): ln and exp share one ACT table set.
                nc.scalar.activation(
                    out=rb[64:128, :], in_=rb[64:128, :],
                    func=mybir.ActivationFunctionType.Exp, scale=-1.0)
                ao = aoT_pool.tile([128, 512], F16, tag="aoT", name="aoT")
                for hh in (0, 1):
                    nc.vector.tensor_mul(
                        out=ao[hh * 64:(hh + 1) * 64, :],
                        in0=usb[64:128, hh * 512:(hh + 1) * 512],
                        in1=rb[64:128, hh * 512:(hh + 1) * 512])
                return ao

            def emit_oproj(qb, ao_pairs):
                for qt_l in range(4):
                    qt = 4 * qb + qt_l
                    op = bigp.tile([128, 1024], F32, tag="big", name="op")
                    for half in range(2):
                        for hp in range(HH // 2):
                            nc.tensor.matmul(
                                op[:, half * 512:(half + 1) * 512],
                                lhsT=ao_pairs[hp][:, qt_l * 128:(qt_l + 1) * 128],
                                rhs=wo_sb[hp][:, half * 512:(half + 1) * 512],
                                start=(hp == 0), stop=(hp == 3),
                            )
                    osb = out_pool.tile([128, D], F16, tag="osb", name="osb")
                    nc.vector.tensor_copy(out=osb[:], in_=op[:])
                    nc.sync.dma_start(
                        out=out[qt * 128:(qt + 1) * 128, :], in_=osb[:])

            # Demand-driven schedule: Q/K chains for token-block tbb=0 are
            # dq-interleaved up front (pair (0,0) unblocks after two chains);
            # chains for later tbb are emitted inside the preceding block's
            # pairs; V chains just before the block needing them;
            # out-projections three pairs after their block.
            emit_qk_chain(0, 0, False)
            emit_qk_chain(0, 0, True)
            for dq in range(1, HD // 128):
                emit_qk_chain(0, dq, False)
                emit_qk_chain(0, dq, True)
            for tb in range(4):
                emit_v_chain(tb)
            pairs = [(qb, hp) for qb in range(n_qb) for hp in range(HH // 2)]
            pT_next = {}
            emit_pair(None, pairs[0], None, pT_next)
            ao_by_qb = {qb: [] for qb in range(n_qb)}
            oproj_queue = []
            for i, (qb, hp) in enumerate(pairs):
                pT_cur, pT_next = pT_next, {}
                nxt = pairs[i + 1] if i + 1 < len(pairs) else None
                if nxt and nxt[1] == 0 and nxt[0] > 0:
                    for tb in range(4 * nxt[0], 4 * nxt[0] + 4):
                        emit_v_chain(tb)
                if oproj_queue and oproj_queue[0][1] <= i:
                    oqb, _ = oproj_queue.pop(0)
                    emit_oproj(oqb, ao_by_qb.pop(oqb))
                ao_by_qb[qb].append(emit_pair((qb, hp), nxt, pT_cur, pT_next))
                # Q/K chains for the next 1024-token block, two chains per
                # pair, emitted during the even q-block preceding it.
                chain_tbb = qb // 2 + 1
                if qb % 2 == 0 and chain_tbb < n_tbb:
                    emit_qk_chain(chain_tbb, hp, False)
                    emit_qk_chain(chain_tbb, hp, True)
                if hp == HH // 2 - 1:
                    oproj_queue.append((qb, i + 3))
            for oqb, _ in oproj_queue:
                emit_oproj(oqb, ao_by_qb.pop(oqb))

    nc.compile()
    return nc


_NC_CACHE = {}


def _get_nc(s=S):
    if s not in _NC_CACHE:
        _NC_CACHE[s] = build_nc(s)
    return _NC_CACHE[s]


def make_in_maps(x, w_q, w_k, w_v, w_o, s=S):
    """Host-side sharding: returns the 8 per-core input maps."""
    x = np.ascontiguousarray(np.asarray(x, dtype=np.float32))
    w_q = np.asarray(w_q, dtype=np.float32)
    w_k = np.asarray(w_k, dtype=np.float32)
    w_v = np.asarray(w_v, dtype=np.float32)
    w_o = np.asarray(w_o, dtype=np.float32)

    xTs = [np.ascontiguousarray(x[b].T.astype(np.float16)) for b in range(B)]
    wqTs = [np.ascontiguousarray(w_q[hg * HD:(hg + 1) * HD, :].T.astype(np.float16)) for hg in range(2)]
    wkTs = [np.ascontiguousarray(w_k[hg * HD:(hg + 1) * HD, :].T.astype(np.float16)) for hg in range(2)]
    wvTs = [np.ascontiguousarray(w_v[hg * HD:(hg + 1) * HD, :].T.astype(np.float16)) for hg in range(2)]
    woTs = [np.ascontiguousarray(w_o[:, hg * HD:(hg + 1) * HD].T.astype(np.float16)) for hg in range(2)]

    in_maps = []
    for c in range(N_CORES):
        b, hg = c // 2, c % 2
        in_maps.append({
            "xT": xTs[b], "wqT": wqTs[hg], "wkT": wkTs[hg],
            "wvT": wvTs[hg], "woT": woTs[hg],
        })
    return in_maps


def kernel(x, w_q, w_k, w_v, w_o, b_o):
    nc = _get_nc(S)
    in_maps = make_in_maps(x, w_q, w_k, w_v, w_o, s=S)
    res = run_bass_kernel_spmd(nc, in_maps, core_ids=list(range(N_CORES)))
    b_o = np.asarray(b_o, dtype=np.float32)
    outp = np.empty((B, S, D), dtype=np.float32)
    for b in range(B):
        outp[b] = (res.results[2 * b]["out"].astype(np.float32)
                   + res.results[2 * b + 1]["out"].astype(np.float32) + b_o)
    return outp
